# revision 32
# baseline (speedup 1.0000x reference)
"""Multi-head self-attention on 8 Trainium2 NeuronCores.

Sharding: tensor-parallel over heads (2 heads per core, both batch elements
on every core). Each core computes qkv projection / attention / its slice of
the output projection (rows of W_out for its heads), producing a partial
[B, N, D] output (bf16); the host sums the 8 partials and adds b_out.

Per-core dataflow (layouts chosen so no engine ever needs a cross-partition
shift except via DMA):
  - host supplies x^T quarter-major [B, 4, 128, DCX*512] so each 512-column
    quarter is one contiguous-per-partition (8 KB) DMA; W_qkv is eb-major
    [3, 128, DCX*128] for the same reason. 12 large DMAs total, issued from
    three engines in parallel (sync / vector / scalar), replace ~100 small
    ones whose ~0.6 us serial issue cost dominated the old startup.
  - QKV^T = Wsel^T @ x^T -> Q^T, K^T, V^T, each [128=2*64 head rows, N]
  - V^T is PE-transposed back to V [k, e] chunks with a LEADING ones column
    per head so the P@V matmul also produces the softmax row-sums, landing
    on psum partition 0 where the normalize chain can read them directly
  - S^T = K^T(head)^T-block @ Q^T (contraction = head dim 64). The two
    heads' S matmuls write one shared [128, 2, 512] psum tile and carry
    disjoint PE row-tiles (auto tile_position (0,0)/(64,0)), so the
    hardware runs them concurrently and one exp covers both heads.
  - P^T = exp(S^T / sqrt(dp)) fused in the PSUM->SBUF evacuation on ScalarE
    (no max subtraction: scores are ~N(0,1), exp is safe in fp32)
  - O^T_aug = [1|V]^T-block @ P^T -> row 0 = softmax denominator s, rows
    1..64 = unnormalized O^T; normalize via fast-approx reciprocal + gpsimd
    partition_broadcast + multiply, DMA-shift into the combined O^T tile
  - y_partial = O^T-block^T @ W_out_slice

The attention phase is one flat stream of 128 windows (b, qh, kc). Window
order on PE: S(W) FIRST (so exp(W) is never blocked behind filler work),
then PV(W-2), then deadline/filler pops. The PV drain and normalization of
a qh spill past its boundary into the next qh's early windows, so the exp
chain on ScalarE (the pacing engine: 128 x ~1.15us) stays saturated.
Deferred QKV prep is emitted in 256-column quarters (~0.85 us of PE work)
so a single pop never overruns the per-window PE budget by more than the
one window of elasticity the 2-deep S psum ring provides.

The PE clock gate (HAM) only counts real matmuls as activity, so warmup is
~20 junk 128x128 matmuls on the identity (issued while the first DMA chunks
stream in), and a few more junk matmuls after the last window keep the
clock warm through the final normalize -> projection -> DMA drain.

QCH=512 keeps every psum user at 1-2 banks: 4 banks S double-buffer +
2 banks PV accumulators + 2 banks for filler work (projection blocks and
the deferred QKV pieces), so fillers never contend with the S ring.
"""

import numpy as np
import ml_dtypes

B, N, D, H, DP = 2, 2048, 1024, 16, 64
SCALE = float(DP) ** 0.5
NCORES = 8
HC = H // NCORES            # heads per core = 2
E = HC * DP                 # per-core head-dim total = 128
QCH = 512                   # q columns handled per attention chunk
NQ = N // QCH               # 4
KB = N // 128               # 16 k blocks
DC = D // 128               # 8 contraction chunks for the qkv projection
NW = B * NQ * KB            # 128 attention windows

BF16 = ml_dtypes.bfloat16

_CACHE = {}


def _build_bass(with_bias=False):
    import concourse.bass as bass
    import concourse.mybir as mybir
    import concourse.tile as tile
    from concourse import bacc
    from concourse.masks import make_identity

    MM_DT = mybir.dt.bfloat16    # matmul input dtype
    P_DT = mybir.dt.bfloat16     # exp(S^T) storage dtype
    F32 = mybir.dt.float32

    # nonzero b_qkv is handled by an extra contraction chunk whose x^T rows
    # are [ones, 0...] and whose weight rows carry the bias (bias as matmul)
    DCX = DC + (1 if with_bias else 0)
    VAW = 130  # VA free width: 2 heads x [ones | V(64)]
    RING = 4   # P^T ring depth (PV runs at lag 2)
    nc = bacc.Bacc(None, target_bir_lowering=False)
    # x^T quarter-major: [B, quarter, partition, dc*512] (8KB/partition/DMA)
    xt = nc.dram_tensor("xt", [B, NQ, 128, DCX * 512], MM_DT, kind="ExternalInput")[:]
    # W_qkv partition-major: [partition, eb*dc*128] -> ONE 6KB-run transfer
    wsel = nc.dram_tensor("wsel", [128, 3 * DCX * 128], MM_DT, kind="ExternalInput")[:]
    wout = nc.dram_tensor("wout", [E, D], MM_DT, kind="ExternalInput")[:]
    # bf16 partials halve the output DMA; the host sums in fp32
    y = nc.dram_tensor("y", [B, N, D], MM_DT, kind="ExternalOutput")[:]

    with tile.TileContext(nc) as tc:
        with (
            tc.tile_pool(name="consts", bufs=1) as consts,
            tc.tile_pool(name="xtp", bufs=2) as xtp,
            tc.tile_pool(name="ptp", bufs=4) as ptp,
            tc.tile_pool(name="qkvp", bufs=2) as qkvp,
            tc.tile_pool(name="vap", bufs=2) as vap,
            tc.tile_pool(name="otp", bufs=2) as otp,
            tc.tile_pool(name="evacp", bufs=2) as evacp,
            tc.tile_pool(name="normp", bufs=2) as normp,
            # 8 psum banks total: paired-S 2x[128,2,512]f32 (4) +
            # pv accumulators 2x[65,512] (2) + filler scratch 2x[128,512] (2)
            tc.tile_pool(name="ps_s", bufs=2, space="PSUM") as ps_s,
            tc.tile_pool(name="ps_g", bufs=2, space="PSUM") as ps_g,
            tc.tile_pool(name="ps_y", bufs=2, space="PSUM") as ps_y,
        ):
            # SBUF layouts mirror the dram layouts (x^T quarter-major, W
            # eb-major) so every transfer is contiguous on BOTH sides:
            # 8KB runs per partition hit ~340 GB/s vs ~105 GB/s for the
            # 1KB-descriptor-run strided version.
            XTs = []
            for b in range(B):
                XTs.append(xtp.tile([128, NQ, DCX, 512], MM_DT, tag="xt", name="xt"))
            WS = consts.tile([128, 3, DCX, E], MM_DT)
            WOUT = consts.tile([128, D], MM_DT)

            def xt_dma(eng, b2, q):
                # full quarters only: slicing the free axis would split the
                # 8KB-contiguous runs into sub-1KB descriptors and drop the
                # transfer from ~270 GB/s to ~100 GB/s
                eng.dma_start(
                    out=XTs[b2][:, q],
                    in_=xt[b2, q].rearrange("p (dc n) -> p dc n", dc=DCX),
                )

            # Three independent DMA paths exist (sync HWDGE ring, scalar
            # HWDGE ring, gpsimd SWDGE); transfers FIFO-serialize per ring,
            # so the first S window's inputs are spread across all three:
            # scalar carries x quarter 0 + W_k, sync W_q, gpsimd W_v. The
            # WARM exp (which pulls in the ~1.3us exp table load) is
            # emitted mid-ring so the table is resident by S(0).
            # HBM bandwidth (~358 GB/s) is shared by all in-flight
            # transfers, so batch b1's 4MB rides BEHIND b0's quarters in
            # the same FIFO rings — b0 (all the first windows need) gets
            # the full bandwidth and lands ~13us instead of ~25us.
            WARM = consts.tile([1, 1], F32)
            nc.vector.memset(WARM, 0.0)
            wsr = wsel.rearrange("p (e dc f) -> p e dc f", e=3, dc=DCX)
            nc.sync.dma_start(out=WS[:, 0:2], in_=wsr[:, 0:2])   # W_q + W_k
            nc.sync.dma_start(out=WS[:, 2:3], in_=wsr[:, 2:3])   # W_v (late ok)
            xt_dma(nc.scalar, 0, 0)  # x b0 quarter 0
            nc.scalar.activation(
                out=WARM, in_=WARM, func=mybir.ActivationFunctionType.Exp
            )
            xt_dma(nc.scalar, 0, 1)
            xt_dma(nc.scalar, 0, 2)
            xt_dma(nc.scalar, 0, 3)
            nc.sync.dma_start(out=WOUT, in_=wout)
            # batch b1's 4MB is deliberately NOT issued here: it would
            # steal HBM bandwidth from b0's quarters (the whole startup
            # critical path). It is issued on gpsimd after the first
            # normalize (~28us), still ~50us before its first reader.

            IDENT = consts.tile([128, 128], MM_DT)
            make_identity(nc, IDENT)
            # HAM warmup: the PE clock gate only counts real matmuls (not
            # transposes) as activity. ~80 junk matmuls on the identity
            # bridge the DMA wait so the real prep matmuls run at 2.4 GHz.
            WARMPS = ps_g.tile([128, 128], F32, tag="g", name="warm_ps")
            for _ in range(60):
                nc.tensor.matmul(WARMPS, lhsT=IDENT, rhs=IDENT, start=True, stop=True)

            QKVTs, VAs = [], []
            fillers = []  # deferred projection sub-blocks (no deadline)
            for b in range(B):
                QKVTs.append(
                    [
                        qkvp.tile([128, N], MM_DT, tag=f"qkv{eb}", name=f"qkv{eb}")
                        for eb in range(3)
                    ]
                )
                # V chunks with a LEADING ones column: [1 | V_h0(64) | 1 | V_h1]
                VA = vap.tile([128, KB, VAW], MM_DT, tag="va", name="va")
                nc.gpsimd.memset(VA[:, :, 0:1], 1.0)
                nc.gpsimd.memset(VA[:, :, VAW // 2 : VAW // 2 + 1], 1.0)
                VAs.append(VA)

            def emit_qkv_quarter(b2, eb, qq, pool, split=False):
                # quarter of a projection block: one 256-col chunk of QKV^T.
                # split=True emits only the first half of the contraction
                # (4 matmuls ~ 0.43us, inside one window's PE slack) and
                # returns a continuation for the rest; the caller must run
                # it before any other allocation from the same psum ring.
                tag = "g" if pool is ps_g else "q"
                ps = pool.tile([128, 256], F32, tag=tag, name="ps_qkv")
                c0 = qq * 256
                qtr, o = qq // 2, (qq % 2) * 256

                def run(dcs, fin):
                    for dc in dcs:
                        nc.tensor.matmul(
                            ps,
                            lhsT=WS[:, eb, dc, :],
                            rhs=XTs[b2][:, qtr, dc, o : o + 256],
                            start=(dc == 0),
                            stop=(dc == DCX - 1),
                        )
                    if fin:
                        nc.vector.tensor_copy(
                            out=QKVTs[b2][eb][:, c0 : c0 + 256], in_=ps
                        )

                if split:
                    run(range(4), False)
                    return lambda: run(range(4, DCX), True)
                run(range(DCX), True)
                return None

            def emit_vtrans(b2, kc, pool):
                tag = "g" if pool is ps_g else "q"
                pst = pool.tile([128, 128], MM_DT, tag=tag, name="ps_vt")
                VT2 = QKVTs[b2][2]
                VA2 = VAs[b2]
                nc.tensor.transpose(
                    pst, VT2[:, kc * 128 : (kc + 1) * 128], IDENT
                )
                nc.vector.tensor_copy(out=VA2[:, kc, 1 : 1 + DP], in_=pst[:, 0:DP])
                nc.vector.tensor_copy(
                    out=VA2[:, kc, VAW // 2 + 1 : VAW // 2 + 1 + DP],
                    in_=pst[:, DP : 2 * DP],
                )

            # ---- deferred-prep schedule. Window index W counts kc windows
            # globally (16 per qh, 64 per batch). Each prep item carries the
            # last window index at which it may be emitted (one before its
            # first reader); pops happen after S(W)/PV(W-2), and the reader
            # S(W+1) is emitted after the pops of window W, so a deadline of
            # W is safe for readers in window W+1.
            def deadlines(base, items):
                return [(base + dl, it) for dl, it in items]

            # deadlines: S(0) needs only Q quarters 0-1 and K quarter 0, so
            # exactly those go negative (popped in the serial pre-loop);
            # everything else is spread 1-2 items per window just ahead of
            # its first reader: K qq read by S(2qq), V qq by vtrans(2qq)
            # (dl 2qq), vtrans kc by PV(kc) in window kc+2 (dl kc+1).
            prep = []
            for b2 in range(B):
                base = 64 * b2
                items = []
                for qq in range(2 * NQ):
                    qdl = -2 + (qq % 2) if qq < 2 else 16 * (qq // 2) - 6 + (qq % 2)
                    items.append(
                        (qdl,
                         lambda split=False, b2=b2, qq=qq:
                         emit_qkv_quarter(b2, 0, qq, ps_y, split))
                    )
                    items.append(
                        (-2 if qq == 0 else 2 * qq - 1,
                         lambda split=False, b2=b2, qq=qq:
                         emit_qkv_quarter(b2, 1, qq, ps_y, split))
                    )
                    items.append(
                        (2 * qq,
                         lambda split=False, b2=b2, qq=qq:
                         emit_qkv_quarter(b2, 2, qq, ps_y, split))
                    )
                for kc in range(KB):
                    items.append(
                        (kc + 1,
                         lambda split=False, b2=b2, kc=kc:
                         emit_vtrans(b2, kc, ps_y))
                    )
                prep.extend(deadlines(base, items))
            prep.sort(key=lambda it: it[0])

            # b=0 items that must precede S(0) run serially now
            while prep and prep[0][0] < 0:
                _, it = prep.pop(0)
                it()

            def emit_proj_block(spec, pool=None, use_scalar=False):
                b2, OT2, nb = spec
                pool = ps_y if pool is None else pool
                tag = "g" if pool is ps_g else ("s" if pool is ps_s else "q")
                ysb = evacp.tile([128, D], MM_DT, tag="y", name="ysb", bufs=4)
                for dc2 in range(D // 512):
                    py = pool.tile([128, 512], F32, tag=tag, name="py")
                    nc.tensor.matmul(
                        py,
                        lhsT=OT2[:, nb * 128 : (nb + 1) * 128],
                        rhs=WOUT[:, dc2 * 512 : (dc2 + 1) * 512],
                        start=True,
                        stop=True,
                    )
                    if use_scalar and dc2 == 1:
                        # drain phase only: ScalarE is done with exps, so
                        # splitting the evacuation halves the serial DVE
                        # cast chain on the critical tail
                        nc.scalar.activation(
                            out=ysb[:, dc2 * 512 : (dc2 + 1) * 512],
                            in_=py,
                            func=mybir.ActivationFunctionType.Copy,
                        )
                    else:
                        nc.vector.tensor_copy(
                            out=ysb[:, dc2 * 512 : (dc2 + 1) * 512], in_=py
                        )
                nc.sync.dma_start(
                    out=y[b2, nb * 128 : (nb + 1) * 128, :], in_=ysb
                )

            # ---- phase 2: attention, one flat stream of NW windows.
            OTs = [otp.tile([128, N], MM_DT, tag="ot", name="ot") for b in range(B)]
            PTmap = {}
            pvmap = {}

            def pv_mms(W2):
                b, qh, kc = W2 // 64, (W2 // 16) % 4, W2 % 16
                if kc == 0:
                    # allocate at first write, strictly after ALL of the
                    # previous qh's accesses to the evicted ring slots have
                    # been emitted (PV tail + normalize reads) — allocating
                    # any earlier races the ring reuse
                    pvmap[(b, qh)] = [
                        ps_g.tile([DP + 1, 512], F32, tag="g", name=f"pv{h}")
                        for h in range(HC)
                    ]
                pvs = pvmap[(b, qh)]
                PT = PTmap[(b, qh)]
                VA = VAs[b]
                for h in range(HC):
                    nc.tensor.matmul(
                        pvs[h],
                        lhsT=VA[
                            :, kc,
                            h * (VAW // 2) : h * (VAW // 2) + DP + 1,
                        ],
                        rhs=PT[:, kc % RING, h, :],
                        start=(kc == 0),
                        stop=(kc == KB - 1),
                    )

            def emit_norm(b, qh, use_scalar=False):
                # normalize: denominator row is psum partition 0 (leading
                # ones column). reciprocal straight off the psum denominator
                # row FIRST (the gpsimd broadcast then runs concurrently
                # with the pv evacuation copy), then broadcast, multiply
                # (rows 0..64 for base-partition alignment; row 0 unused),
                # DMA-shift into O^T.
                pvs = pvmap.pop((b, qh))
                OT = OTs[b]
                for h in range(HC):
                    pv = pvs[h]
                    rt = normp.tile([1, QCH], F32, tag="rt", name="rt")
                    nc.vector.reciprocal_approx_fast(out=rt, in_=pv[0:1, :])
                    ocp = normp.tile([DP + 1, QCH], F32, tag="ocp", name="ocp")
                    nc.vector.tensor_copy(out=ocp, in_=pv)
                    bc = normp.tile([DP + 1, QCH], F32, tag="bc", name="bc")
                    nc.gpsimd.partition_broadcast(bc, rt)
                    ots = normp.tile([DP + 1, QCH], MM_DT, tag="ots", name="ots")
                    nc.vector.tensor_mul(out=ots, in0=ocp, in1=bc)
                    nc.sync.dma_start(
                        out=OT[h * DP : (h + 1) * DP, qh * QCH : (qh + 1) * QCH],
                        in_=ots[1 : DP + 1, :],
                    )
                # queue this qh's projection blocks as fillers
                for nb in range(qh * QCH // 128, (qh + 1) * QCH // 128):
                    fillers.append(
                        (lambda pool=None, use_scalar=False, s=(b, OT, nb):
                         emit_proj_block(s, pool, use_scalar))
                    )
                if (b, qh) == (0, 0):
                    # b0 is resident; start streaming b1 now
                    for q2 in range(NQ):
                        xt_dma(nc.gpsimd, 1, q2)

            pending = [None]
            for W in range(NW + 2):
                b, qh, kc = W // 64, (W // 16) % 4, W % 16
                if W < NW:
                    if kc == 0:
                        PTmap[(b, qh)] = ptp.tile(
                            [128, RING, HC, 512], P_DT, tag="pt", name="pt"
                        )
                    # S FIRST: exp(W) is gated only on exp(W-1) and S(W),
                    # so nothing else may sit between them on the PE
                    PT = PTmap[(b, qh)]
                    QT, KT = QKVTs[b][0], QKVTs[b][1]
                    ps2 = ps_s.tile([128, HC, 512], F32, tag="s", name="s2")
                    q0 = qh * QCH
                    for h in range(HC):
                        nc.tensor.matmul(
                            ps2[:, h, :],
                            lhsT=KT[
                                h * DP : (h + 1) * DP,
                                kc * 128 : (kc + 1) * 128,
                            ],
                            rhs=QT[h * DP : (h + 1) * DP, q0 : q0 + 512],
                            start=True,
                            stop=True,
                        )
                    nc.scalar.activation(
                        out=PT[:, kc % RING, :, :],
                        in_=ps2,
                        func=mybir.ActivationFunctionType.Exp,
                        scale=1.0 / SCALE,
                    )
                if W >= 2:
                    pv_mms(W - 2)
                    if (W - 2) % 16 == 15:
                        emit_norm((W - 2) // 64, ((W - 2) // 16) % 4,
                                  use_scalar=(W - 2 == NW - 1))
                # pops: a pending half-quarter continuation ALWAYS runs
                # first (it shares a psum tile with its first half, so no
                # other ps_y allocation may intervene), then mandatory
                # deadline pops (emitted whole -- their reader is next
                # window), then ONE opportunistic pop. Opportunistic qkv
                # quarters are split across two windows so a pop never
                # overruns the per-window PE slack.
                popped = False
                if pending[0] is not None:
                    pending[0]()
                    pending[0] = None
                    popped = True
                while prep and prep[0][0] <= W:
                    prep.pop(0)[1]()
                    popped = True
                if not popped and W >= 1:
                    # prep first: its deadlines cluster in each batch's
                    # first qh, so draining it early smooths those windows;
                    # projection fillers have no deadline and soak up the
                    # idle back-half windows instead.
                    if prep and len(fillers) <= 12:
                        pending[0] = prep.pop(0)[1](split=True)
                    elif fillers:
                        fillers.pop(0)(use_scalar=(W >= NW))
                        if len(fillers) > 10:
                            fillers.pop(0)(use_scalar=(W >= NW))

            if pending[0] is not None:
                pending[0]()
                pending[0] = None
            # keep the PE clock warm through the final normalize chain so
            # the last projection blocks run at full clock
            WARMPS2 = ps_y.tile([128, 128], F32, tag="q", name="warm2")
            for _ in range(40):
                nc.tensor.matmul(WARMPS2, lhsT=IDENT, rhs=IDENT, start=True, stop=True)

            # drain remaining fillers, rotating through all three psum
            # pools (the S ring is free once the exps are done) to keep
            # 6 blocks in flight
            di = 0
            drain_pools = [ps_y, ps_g]
            while fillers:
                fillers.pop(0)(drain_pools[di % 2], use_scalar=True)
                di += 1
    nc.finalize()
    return nc


def _get_bass(with_bias=False):
    key = f"nc{int(with_bias)}"
    if key not in _CACHE:
        _CACHE[key] = _build_bass(with_bias)
    return _CACHE[key]


def _make_in_maps(x, W_qkv, b_qkv, W_out):
    """Shard the full inputs into the 8 per-core input dicts."""
    x = np.asarray(x, dtype=np.float32)
    W_qkv = np.asarray(W_qkv, dtype=np.float32)
    b_qkv = np.asarray(b_qkv, dtype=np.float32)
    W_out = np.asarray(W_out, dtype=np.float32)

    with_bias = bool(np.any(b_qkv))
    DCX = DC + (1 if with_bias else 0)
    # x^T per batch, shared by all cores (+ optional bias chunk rows)
    xtt = x.transpose(0, 2, 1)
    if with_bias:
        aug = np.zeros((B, 128, N), dtype=np.float32)
        aug[:, 0, :] = 1.0
        xtt = np.concatenate([xtt, aug], axis=1)
    # quarter-major: [B, 4, 128, DCX*512] so each quarter is one DMA with
    # 8KB contiguous per partition
    xq = (
        xtt.reshape(B, DCX, 128, NQ, 512)
        .transpose(0, 3, 2, 1, 4)
        .reshape(B, NQ, 128, DCX * 512)
    )
    xt = np.ascontiguousarray(xq).astype(BF16)

    in_maps = []
    for c in range(NCORES):
        heads = [HC * c + i for i in range(HC)]
        # W_qkv columns: head h occupies cols [h*3*DP, (h+1)*3*DP) as [q|k|v]
        qcols = [W_qkv[:, h * 3 * DP : h * 3 * DP + DP] for h in heads]
        kcols = [W_qkv[:, h * 3 * DP + DP : h * 3 * DP + 2 * DP] for h in heads]
        vcols = [W_qkv[:, h * 3 * DP + 2 * DP : h * 3 * DP + 3 * DP] for h in heads]
        wsel = np.concatenate(qcols + kcols + vcols, axis=1)  # [D, 3*E]
        if with_bias:
            bq = [b_qkv[h * 3 * DP : h * 3 * DP + DP] for h in heads]
            bk = [b_qkv[h * 3 * DP + DP : h * 3 * DP + 2 * DP] for h in heads]
            bv = [b_qkv[h * 3 * DP + 2 * DP : h * 3 * DP + 3 * DP] for h in heads]
            brow = np.concatenate(bq + bk + bv)  # [3*E]
            baug = np.zeros((128, 3 * E), dtype=np.float32)
            baug[0, :] = brow
            wsel = np.concatenate([wsel, baug], axis=0)
        # partition-major [128, 3*DCX*128]: one 6KB-contiguous-run transfer
        wq = (
            wsel.reshape(DCX, 128, 3, E)
            .transpose(1, 2, 0, 3)
            .reshape(128, 3 * DCX * E)
        )
        woutc = np.concatenate(
            [W_out[h * DP : (h + 1) * DP, :] for h in heads], axis=0
        )  # [E, D]
        in_maps.append(
            {
                "xt": xt,
                "wsel": np.ascontiguousarray(wq).astype(BF16),
                "wout": np.ascontiguousarray(woutc).astype(BF16),
            }
        )
    return in_maps, with_bias


def _run(in_maps, with_bias=False, trace=False):
    from concourse import bass_utils

    nc = _get_bass(with_bias)
    return bass_utils.run_bass_kernel_spmd(
        nc, in_maps, core_ids=list(range(NCORES)), trace=trace
    )


def kernel(x, W_qkv, b_qkv, W_out, b_out, _trace=False):
    in_maps, with_bias = _make_in_maps(x, W_qkv, b_qkv, W_out)
    res = _run(in_maps, with_bias=with_bias, trace=_trace)
    y = np.zeros((B, N, D), dtype=np.float32)
    for r in res.results:
        y += np.asarray(r["y"], dtype=np.float32)
    y += np.asarray(b_out, dtype=np.float32)
    _CACHE["last_result"] = res
    return y


# revision 33
# speedup vs baseline: 1.0651x; 1.0651x over previous
"""Multi-head self-attention on 8 Trainium2 NeuronCores.

Sharding: tensor-parallel over heads (2 heads per core, both batch elements
on every core). Each core computes qkv projection / attention / its slice of
the output projection (rows of W_out for its heads), producing a partial
[B, N, D] output (bf16); the host sums the 8 partials and adds b_out.

Per-core dataflow (layouts chosen so no engine ever needs a cross-partition
shift except via DMA):
  - host supplies x^T quarter-major [B, 4, 128, DCX*512] so each 512-column
    quarter is one contiguous-per-partition (8 KB) DMA; W_qkv is eb-major
    [3, 128, DCX*128] for the same reason. 12 large DMAs total, issued from
    three engines in parallel (sync / vector / scalar), replace ~100 small
    ones whose ~0.6 us serial issue cost dominated the old startup.
  - QKV^T = Wsel^T @ x^T -> Q^T, K^T, V^T, each [128=2*64 head rows, N]
  - V^T is PE-transposed back to V [k, e] chunks with a LEADING ones column
    per head so the P@V matmul also produces the softmax row-sums, landing
    on psum partition 0 where the normalize chain can read them directly
  - S^T = K^T(head)^T-block @ Q^T (contraction = head dim 64). The two
    heads' S matmuls write one shared [128, 2, 512] psum tile and carry
    disjoint PE row-tiles (auto tile_position (0,0)/(64,0)), so the
    hardware runs them concurrently and one exp covers both heads.
  - P^T = exp(S^T / sqrt(dp)) fused in the PSUM->SBUF evacuation on ScalarE
    (no max subtraction: scores are ~N(0,1), exp is safe in fp32)
  - O^T_aug = [1|V]^T-block @ P^T -> row 0 = softmax denominator s, rows
    1..64 = unnormalized O^T; normalize via fast-approx reciprocal + gpsimd
    partition_broadcast + multiply, DMA-shift into the combined O^T tile
  - y_partial = O^T-block^T @ W_out_slice

The attention phase is one flat stream of 128 windows (b, qh, kc). Window
order on PE: S(W) FIRST (so exp(W) is never blocked behind filler work),
then PV(W-2), then deadline/filler pops. The PV drain and normalization of
a qh spill past its boundary into the next qh's early windows, so the exp
chain on ScalarE (the pacing engine: 128 x ~1.15us) stays saturated.
Deferred QKV prep is emitted in 256-column quarters (~0.85 us of PE work)
so a single pop never overruns the per-window PE budget by more than the
one window of elasticity the 2-deep S psum ring provides.

The PE clock gate (HAM) only counts real matmuls as activity, so warmup is
~20 junk 128x128 matmuls on the identity (issued while the first DMA chunks
stream in), and a few more junk matmuls after the last window keep the
clock warm through the final normalize -> projection -> DMA drain.

QCH=512 keeps every psum user at 1-2 banks: 4 banks S double-buffer +
2 banks PV accumulators + 2 banks for filler work (projection blocks and
the deferred QKV pieces), so fillers never contend with the S ring.
"""

import numpy as np
import ml_dtypes

B, N, D, H, DP = 2, 2048, 1024, 16, 64
SCALE = float(DP) ** 0.5
NCORES = 8
HC = H // NCORES            # heads per core = 2
E = HC * DP                 # per-core head-dim total = 128
QCH = 512                   # q columns handled per attention chunk
NQ = N // QCH               # 4
KB = N // 128               # 16 k blocks
DC = D // 128               # 8 contraction chunks for the qkv projection
NW = B * NQ * KB            # 128 attention windows

BF16 = ml_dtypes.bfloat16

_CACHE = {}


def _build_bass(with_bias=False):
    import concourse.bass as bass
    import concourse.mybir as mybir
    import concourse.tile as tile
    from concourse import bacc
    from concourse.masks import make_identity

    MM_DT = mybir.dt.bfloat16    # matmul input dtype
    P_DT = mybir.dt.bfloat16     # exp(S^T) storage dtype
    F32 = mybir.dt.float32

    # nonzero b_qkv is handled by an extra contraction chunk whose x^T rows
    # are [ones, 0...] and whose weight rows carry the bias (bias as matmul)
    DCX = DC + (1 if with_bias else 0)
    VAW = 130  # VA free width: 2 heads x [ones | V(64)]
    RING = 4   # P^T ring depth (PV runs at lag 2)
    nc = bacc.Bacc(None, target_bir_lowering=False)
    # x^T quarter-major: [B, quarter, partition, dc*512] (8KB/partition/DMA)
    xt = nc.dram_tensor("xt", [B, NQ, 128, DCX * 512], MM_DT, kind="ExternalInput")[:]
    # W_qkv partition-major: [partition, eb*dc*128] -> ONE 6KB-run transfer
    wsel = nc.dram_tensor("wsel", [128, 3 * DCX * 128], MM_DT, kind="ExternalInput")[:]
    wout = nc.dram_tensor("wout", [E, D], MM_DT, kind="ExternalInput")[:]
    # bf16 partials halve the output DMA; the host sums in fp32
    y = nc.dram_tensor("y", [B, N, D], MM_DT, kind="ExternalOutput")[:]

    with tile.TileContext(nc) as tc:
        with (
            tc.tile_pool(name="consts", bufs=1) as consts,
            tc.tile_pool(name="xtp", bufs=2) as xtp,
            tc.tile_pool(name="ptp", bufs=4) as ptp,
            tc.tile_pool(name="qkvp", bufs=2) as qkvp,
            tc.tile_pool(name="vap", bufs=2) as vap,
            tc.tile_pool(name="otp", bufs=2) as otp,
            tc.tile_pool(name="evacp", bufs=2) as evacp,
            tc.tile_pool(name="normp", bufs=2) as normp,
            # 8 psum banks total: paired-S 2x[128,2,512]f32 (4) +
            # pv accumulators 2x[65,512] (2) + filler scratch 2x[128,512] (2)
            tc.tile_pool(name="ps_s", bufs=2, space="PSUM") as ps_s,
            tc.tile_pool(name="ps_g", bufs=2, space="PSUM") as ps_g,
            tc.tile_pool(name="ps_y", bufs=2, space="PSUM") as ps_y,
        ):
            # SBUF layouts mirror the dram layouts (x^T quarter-major, W
            # eb-major) so every transfer is contiguous on BOTH sides:
            # 8KB runs per partition hit ~340 GB/s vs ~105 GB/s for the
            # 1KB-descriptor-run strided version.
            XTs = []
            for b in range(B):
                XTs.append(xtp.tile([128, NQ, DCX, 512], MM_DT, tag="xt", name="xt"))
            WS = consts.tile([128, 3, DCX, E], MM_DT)
            WOUT = consts.tile([128, D], MM_DT)

            def xt_dma(eng, b2, q):
                # full quarters only: slicing the free axis would split the
                # 8KB-contiguous runs into sub-1KB descriptors and drop the
                # transfer from ~270 GB/s to ~100 GB/s
                eng.dma_start(
                    out=XTs[b2][:, q],
                    in_=xt[b2, q].rearrange("p (dc n) -> p dc n", dc=DCX),
                )

            # Three independent DMA paths exist (sync HWDGE ring, scalar
            # HWDGE ring, gpsimd SWDGE); transfers FIFO-serialize per ring,
            # so the first S window's inputs are spread across all three:
            # scalar carries x quarter 0 + W_k, sync W_q, gpsimd W_v. The
            # WARM exp (which pulls in the ~1.3us exp table load) is
            # emitted mid-ring so the table is resident by S(0).
            # HBM bandwidth (~358 GB/s) is shared by all in-flight
            # transfers, so batch b1's 4MB rides BEHIND b0's quarters in
            # the same FIFO rings — b0 (all the first windows need) gets
            # the full bandwidth and lands ~13us instead of ~25us.
            WARM = consts.tile([1, 1], F32)
            nc.vector.memset(WARM, 0.0)
            wsr = wsel.rearrange("p (e dc f) -> p e dc f", e=3, dc=DCX)
            nc.sync.dma_start(out=WS[:, 0:2], in_=wsr[:, 0:2])   # W_q + W_k
            nc.sync.dma_start(out=WS[:, 2:3], in_=wsr[:, 2:3])   # W_v (late ok)
            xt_dma(nc.scalar, 0, 0)  # x b0 quarter 0
            nc.scalar.activation(
                out=WARM, in_=WARM, func=mybir.ActivationFunctionType.Exp
            )
            xt_dma(nc.scalar, 0, 1)
            xt_dma(nc.scalar, 0, 2)
            xt_dma(nc.scalar, 0, 3)
            nc.sync.dma_start(out=WOUT, in_=wout)
            # batch b1's 4MB is deliberately NOT issued here: it would
            # steal HBM bandwidth from b0's quarters (the whole startup
            # critical path). It is issued on gpsimd after the first
            # normalize (~28us), still ~50us before its first reader.

            IDENT = consts.tile([128, 128], MM_DT)
            make_identity(nc, IDENT)
            # HAM warmup: the PE clock gate only counts real matmuls (not
            # transposes) as activity. ~80 junk matmuls on the identity
            # bridge the DMA wait so the real prep matmuls run at 2.4 GHz.
            WARMPS = ps_g.tile([128, 128], F32, tag="g", name="warm_ps")
            for _ in range(60):
                nc.tensor.matmul(WARMPS, lhsT=IDENT, rhs=IDENT, start=True, stop=True)

            QKVTs, VAs = [], []
            fillers = []  # deferred projection sub-blocks (no deadline)
            for b in range(B):
                QKVTs.append(
                    [
                        qkvp.tile([128, N], MM_DT, tag=f"qkv{eb}", name=f"qkv{eb}")
                        for eb in range(3)
                    ]
                )
                # V chunks with a LEADING ones column: [1 | V_h0(64) | 1 | V_h1]
                VA = vap.tile([128, KB, VAW], MM_DT, tag="va", name="va")
                nc.gpsimd.memset(VA[:, :, 0:1], 1.0)
                nc.gpsimd.memset(VA[:, :, VAW // 2 : VAW // 2 + 1], 1.0)
                VAs.append(VA)

            def emit_qkv_quarter(b2, eb, qq, pool, split=False):
                # quarter of a projection block: one 256-col chunk of QKV^T.
                # split=True emits only the first half of the contraction
                # (4 matmuls ~ 0.43us, inside one window's PE slack) and
                # returns a continuation for the rest; the caller must run
                # it before any other allocation from the same psum ring.
                tag = "g" if pool is ps_g else "q"
                ps = pool.tile([128, 256], F32, tag=tag, name="ps_qkv")
                c0 = qq * 256
                qtr, o = qq // 2, (qq % 2) * 256

                def run(dcs, fin):
                    for dc in dcs:
                        nc.tensor.matmul(
                            ps,
                            lhsT=WS[:, eb, dc, :],
                            rhs=XTs[b2][:, qtr, dc, o : o + 256],
                            start=(dc == 0),
                            stop=(dc == DCX - 1),
                        )
                    if fin:
                        nc.vector.tensor_copy(
                            out=QKVTs[b2][eb][:, c0 : c0 + 256], in_=ps
                        )

                if split:
                    run(range(4), False)
                    return lambda: run(range(4, DCX), True)
                run(range(DCX), True)
                return None

            def emit_vtrans(b2, kc, pool):
                tag = "g" if pool is ps_g else "q"
                pst = pool.tile([128, 128], MM_DT, tag=tag, name="ps_vt")
                VT2 = QKVTs[b2][2]
                VA2 = VAs[b2]
                nc.tensor.transpose(
                    pst, VT2[:, kc * 128 : (kc + 1) * 128], IDENT
                )
                nc.vector.tensor_copy(out=VA2[:, kc, 1 : 1 + DP], in_=pst[:, 0:DP])
                nc.vector.tensor_copy(
                    out=VA2[:, kc, VAW // 2 + 1 : VAW // 2 + 1 + DP],
                    in_=pst[:, DP : 2 * DP],
                )

            # ---- deferred-prep schedule. Window index W counts kc windows
            # globally (16 per qh, 64 per batch). Each prep item carries the
            # last window index at which it may be emitted (one before its
            # first reader); pops happen after S(W)/PV(W-2), and the reader
            # S(W+1) is emitted after the pops of window W, so a deadline of
            # W is safe for readers in window W+1.
            def deadlines(base, items):
                return [(base + dl, it) for dl, it in items]

            # deadlines: S(0) needs only Q quarters 0-1 and K quarter 0, so
            # exactly those go negative (popped in the serial pre-loop);
            # everything else is spread 1-2 items per window just ahead of
            # its first reader: K qq read by S(2qq), V qq by vtrans(2qq)
            # (dl 2qq), vtrans kc by PV(kc) in window kc+2 (dl kc+1).
            prep = []
            for b2 in range(B):
                base = 64 * b2
                items = []
                for qq in range(2 * NQ):
                    qdl = -2 + (qq % 2) if qq < 2 else 16 * (qq // 2) - 6 + (qq % 2)
                    items.append(
                        (qdl,
                         lambda split=False, b2=b2, qq=qq:
                         emit_qkv_quarter(b2, 0, qq, ps_y, split))
                    )
                    items.append(
                        (-2 if qq == 0 else 2 * qq - 1,
                         lambda split=False, b2=b2, qq=qq:
                         emit_qkv_quarter(b2, 1, qq, ps_y, split))
                    )
                    items.append(
                        (2 * qq,
                         lambda split=False, b2=b2, qq=qq:
                         emit_qkv_quarter(b2, 2, qq, ps_y, split))
                    )
                for kc in range(KB):
                    items.append(
                        (kc + 1,
                         lambda split=False, b2=b2, kc=kc:
                         emit_vtrans(b2, kc, ps_y))
                    )
                prep.extend(deadlines(base, items))
            prep.sort(key=lambda it: it[0])

            # b=0 items that must precede S(0) run serially now
            while prep and prep[0][0] < 0:
                _, it = prep.pop(0)
                it()

            def emit_proj_block(spec, pool=None, use_scalar=False):
                b2, OT2, nb = spec
                pool = ps_y if pool is None else pool
                tag = "g" if pool is ps_g else ("s" if pool is ps_s else "q")
                ysb = evacp.tile([128, D], MM_DT, tag="y", name="ysb", bufs=4)
                for dc2 in range(D // 512):
                    py = pool.tile([128, 512], F32, tag=tag, name="py")
                    nc.tensor.matmul(
                        py,
                        lhsT=OT2[:, nb * 128 : (nb + 1) * 128],
                        rhs=WOUT[:, dc2 * 512 : (dc2 + 1) * 512],
                        start=True,
                        stop=True,
                    )
                    if use_scalar and dc2 == 1:
                        # drain phase only: ScalarE is done with exps, so
                        # splitting the evacuation halves the serial DVE
                        # cast chain on the critical tail
                        nc.scalar.activation(
                            out=ysb[:, dc2 * 512 : (dc2 + 1) * 512],
                            in_=py,
                            func=mybir.ActivationFunctionType.Copy,
                        )
                    else:
                        nc.vector.tensor_copy(
                            out=ysb[:, dc2 * 512 : (dc2 + 1) * 512], in_=py
                        )
                nc.sync.dma_start(
                    out=y[b2, nb * 128 : (nb + 1) * 128, :], in_=ysb
                )

            # ---- phase 2: attention, one flat stream of NW windows.
            OTs = [otp.tile([128, N], MM_DT, tag="ot", name="ot") for b in range(B)]
            PTmap = {}
            pvmap = {}

            def pv_mms(W2):
                b, qh, kc = W2 // 64, (W2 // 16) % 4, W2 % 16
                if kc == 0:
                    # allocate at first write, strictly after ALL of the
                    # previous qh's accesses to the evicted ring slots have
                    # been emitted (PV tail + normalize reads) — allocating
                    # any earlier races the ring reuse
                    pvmap[(b, qh)] = [
                        ps_g.tile([DP + 1, 512], F32, tag="g", name=f"pv{h}")
                        for h in range(HC)
                    ]
                pvs = pvmap[(b, qh)]
                PT = PTmap[(b, qh)]
                VA = VAs[b]
                for h in range(HC):
                    nc.tensor.matmul(
                        pvs[h],
                        lhsT=VA[
                            :, kc,
                            h * (VAW // 2) : h * (VAW // 2) + DP + 1,
                        ],
                        rhs=PT[:, kc % RING, h, :],
                        start=(kc == 0),
                        stop=(kc == KB - 1),
                    )

            def emit_norm(b, qh, use_scalar=False):
                # normalize: denominator row is psum partition 0 (leading
                # ones column). reciprocal straight off the psum denominator
                # row FIRST (the gpsimd broadcast then runs concurrently
                # with the pv evacuation copy), then broadcast, multiply
                # (rows 0..64 for base-partition alignment; row 0 unused),
                # DMA-shift into O^T.
                pvs = pvmap.pop((b, qh))
                OT = OTs[b]
                for h in range(HC):
                    pv = pvs[h]
                    rt = normp.tile([1, QCH], F32, tag="rt", name="rt")
                    nc.vector.reciprocal_approx_fast(out=rt, in_=pv[0:1, :])
                    ocp = normp.tile([DP + 1, QCH], F32, tag="ocp", name="ocp")
                    nc.vector.tensor_copy(out=ocp, in_=pv)
                    bc = normp.tile([DP + 1, QCH], F32, tag="bc", name="bc")
                    nc.gpsimd.partition_broadcast(bc, rt)
                    ots = normp.tile([DP + 1, QCH], MM_DT, tag="ots", name="ots")
                    nc.vector.tensor_mul(out=ots, in0=ocp, in1=bc)
                    nc.sync.dma_start(
                        out=OT[h * DP : (h + 1) * DP, qh * QCH : (qh + 1) * QCH],
                        in_=ots[1 : DP + 1, :],
                    )
                # queue this qh's projection blocks as fillers
                for nb in range(qh * QCH // 128, (qh + 1) * QCH // 128):
                    fillers.append(
                        (lambda pool=None, use_scalar=False, s=(b, OT, nb):
                         emit_proj_block(s, pool, use_scalar))
                    )
                if (b, qh) == (0, 0):
                    # b0 is resident; start streaming b1 now
                    for q2 in range(NQ):
                        xt_dma(nc.gpsimd, 1, q2)

            for W in range(NW + 2):
                b, qh, kc = W // 64, (W // 16) % 4, W % 16
                if W < NW:
                    if kc == 0:
                        PTmap[(b, qh)] = ptp.tile(
                            [128, RING, HC, 512], P_DT, tag="pt", name="pt"
                        )
                    # S FIRST: exp(W) is gated only on exp(W-1) and S(W),
                    # so nothing else may sit between them on the PE
                    PT = PTmap[(b, qh)]
                    QT, KT = QKVTs[b][0], QKVTs[b][1]
                    ps2 = ps_s.tile([128, HC, 512], F32, tag="s", name="s2")
                    q0 = qh * QCH
                    for h in range(HC):
                        nc.tensor.matmul(
                            ps2[:, h, :],
                            lhsT=KT[
                                h * DP : (h + 1) * DP,
                                kc * 128 : (kc + 1) * 128,
                            ],
                            rhs=QT[h * DP : (h + 1) * DP, q0 : q0 + 512],
                            start=True,
                            stop=True,
                        )
                    nc.scalar.activation(
                        out=PT[:, kc % RING, :, :],
                        in_=ps2,
                        func=mybir.ActivationFunctionType.Exp,
                        scale=1.0 / SCALE,
                    )
                if W >= 2:
                    pv_mms(W - 2)
                    if (W - 2) % 16 == 15:
                        emit_norm((W - 2) // 64, ((W - 2) // 16) % 4,
                                  use_scalar=(W - 2 == NW - 1))
                # mandatory deadline pops, then ONE opportunistic pop.
                popped = False
                while prep and prep[0][0] <= W:
                    prep.pop(0)[1]()
                    popped = True
                if not popped and W >= 1:
                    # prep first: its deadlines cluster in each batch's
                    # first qh, so draining it early smooths those windows;
                    # projection fillers have no deadline and soak up the
                    # idle back-half windows instead.
                    if prep and len(fillers) <= 12:
                        prep.pop(0)[1]()
                    elif fillers:
                        fillers.pop(0)(use_scalar=(W >= NW))
                        if len(fillers) > 10:
                            fillers.pop(0)(use_scalar=(W >= NW))

            # keep the PE clock warm through the final normalize chain so
            # the last projection blocks run at full clock
            WARMPS2 = ps_y.tile([128, 128], F32, tag="q", name="warm2")
            for _ in range(40):
                nc.tensor.matmul(WARMPS2, lhsT=IDENT, rhs=IDENT, start=True, stop=True)

            # drain remaining fillers, rotating through all three psum
            # pools (the S ring is free once the exps are done) to keep
            # 6 blocks in flight
            di = 0
            drain_pools = [ps_y, ps_g]
            while fillers:
                fillers.pop(0)(drain_pools[di % 2], use_scalar=True)
                di += 1
    nc.finalize()
    return nc


def _get_bass(with_bias=False):
    key = f"nc{int(with_bias)}"
    if key not in _CACHE:
        _CACHE[key] = _build_bass(with_bias)
    return _CACHE[key]


def _make_in_maps(x, W_qkv, b_qkv, W_out):
    """Shard the full inputs into the 8 per-core input dicts."""
    x = np.asarray(x, dtype=np.float32)
    W_qkv = np.asarray(W_qkv, dtype=np.float32)
    b_qkv = np.asarray(b_qkv, dtype=np.float32)
    W_out = np.asarray(W_out, dtype=np.float32)

    with_bias = bool(np.any(b_qkv))
    DCX = DC + (1 if with_bias else 0)
    # x^T per batch, shared by all cores (+ optional bias chunk rows)
    xtt = x.transpose(0, 2, 1)
    if with_bias:
        aug = np.zeros((B, 128, N), dtype=np.float32)
        aug[:, 0, :] = 1.0
        xtt = np.concatenate([xtt, aug], axis=1)
    # quarter-major: [B, 4, 128, DCX*512] so each quarter is one DMA with
    # 8KB contiguous per partition
    xq = (
        xtt.reshape(B, DCX, 128, NQ, 512)
        .transpose(0, 3, 2, 1, 4)
        .reshape(B, NQ, 128, DCX * 512)
    )
    xt = np.ascontiguousarray(xq).astype(BF16)

    in_maps = []
    for c in range(NCORES):
        heads = [HC * c + i for i in range(HC)]
        # W_qkv columns: head h occupies cols [h*3*DP, (h+1)*3*DP) as [q|k|v]
        qcols = [W_qkv[:, h * 3 * DP : h * 3 * DP + DP] for h in heads]
        kcols = [W_qkv[:, h * 3 * DP + DP : h * 3 * DP + 2 * DP] for h in heads]
        vcols = [W_qkv[:, h * 3 * DP + 2 * DP : h * 3 * DP + 3 * DP] for h in heads]
        wsel = np.concatenate(qcols + kcols + vcols, axis=1)  # [D, 3*E]
        if with_bias:
            bq = [b_qkv[h * 3 * DP : h * 3 * DP + DP] for h in heads]
            bk = [b_qkv[h * 3 * DP + DP : h * 3 * DP + 2 * DP] for h in heads]
            bv = [b_qkv[h * 3 * DP + 2 * DP : h * 3 * DP + 3 * DP] for h in heads]
            brow = np.concatenate(bq + bk + bv)  # [3*E]
            baug = np.zeros((128, 3 * E), dtype=np.float32)
            baug[0, :] = brow
            wsel = np.concatenate([wsel, baug], axis=0)
        # partition-major [128, 3*DCX*128]: one 6KB-contiguous-run transfer
        wq = (
            wsel.reshape(DCX, 128, 3, E)
            .transpose(1, 2, 0, 3)
            .reshape(128, 3 * DCX * E)
        )
        woutc = np.concatenate(
            [W_out[h * DP : (h + 1) * DP, :] for h in heads], axis=0
        )  # [E, D]
        in_maps.append(
            {
                "xt": xt,
                "wsel": np.ascontiguousarray(wq).astype(BF16),
                "wout": np.ascontiguousarray(woutc).astype(BF16),
            }
        )
    return in_maps, with_bias


def _run(in_maps, with_bias=False, trace=False):
    from concourse import bass_utils

    nc = _get_bass(with_bias)
    return bass_utils.run_bass_kernel_spmd(
        nc, in_maps, core_ids=list(range(NCORES)), trace=trace
    )


def kernel(x, W_qkv, b_qkv, W_out, b_out, _trace=False):
    in_maps, with_bias = _make_in_maps(x, W_qkv, b_qkv, W_out)
    res = _run(in_maps, with_bias=with_bias, trace=_trace)
    y = np.zeros((B, N, D), dtype=np.float32)
    for r in res.results:
        y += np.asarray(r["y"], dtype=np.float32)
    y += np.asarray(b_out, dtype=np.float32)
    _CACHE["last_result"] = res
    return y


# revision 34
# speedup vs baseline: 1.0770x; 1.0112x over previous
"""Multi-head self-attention on 8 Trainium2 NeuronCores.

Sharding: tensor-parallel over heads (2 heads per core, both batch elements
on every core). Each core computes qkv projection / attention / its slice of
the output projection (rows of W_out for its heads), producing a partial
[B, N, D] output (bf16); the host sums the 8 partials and adds b_out.

Per-core dataflow (layouts chosen so no engine ever needs a cross-partition
shift except via DMA):
  - host supplies x^T quarter-major [B, 4, 128, DCX*512] so each 512-column
    quarter is one contiguous-per-partition (8 KB-run) DMA at ~270 GB/s;
    W_qkv is partition-major [128, 3*DCX*128] for the same reason. ~12
    large DMAs replace ~100 small ones whose ~0.6 us serial issue cost
    dominated the old startup. They are spread over the three DMA-capable
    engine queues (sync / scalar / gpsimd HWDGE+SWDGE rings); batch b1's
    4 MB is issued only after the first normalize so b0's critical
    transfers get the full HBM bandwidth.
  - QKV^T = Wsel^T @ x^T -> Q^T, K^T, V^T, each [128=2*64 head rows, N]
  - V^T is PE-transposed back to V [k, e] chunks with a LEADING ones column
    per head so the P@V matmul also produces the softmax row-sums, landing
    on psum partition 0 where the normalize chain can read them directly
  - S^T = K^T(head)^T-block @ Q^T (contraction = head dim 64). The two
    heads' S matmuls write one shared [128, 2, 512] psum tile and carry
    disjoint PE row-tiles (auto tile_position (0,0)/(64,0)), so the
    hardware runs them concurrently and one exp covers both heads.
  - P^T = exp(S^T / sqrt(dp)) fused in the PSUM->SBUF evacuation on ScalarE
    (no max subtraction: scores are ~N(0,1), exp is safe in fp32)
  - O^T_aug = [1|V]^T-block @ P^T -> row 0 = softmax denominator s, rows
    1..64 = unnormalized O^T; normalize via fast-approx reciprocal + gpsimd
    partition_broadcast + multiply, DMA-shift into the combined O^T tile
  - y_partial = O^T-block^T @ W_out_slice

The attention phase is one flat stream of 128 windows (b, qh, kc). Window
order on PE: S(W) FIRST (so exp(W) is never blocked behind filler work),
then PV(W-2), then deadline/filler pops. The PV drain and normalization of
a qh spill past its boundary into the next qh's early windows, so the exp
chain on ScalarE (the pacing engine: 128 x ~1.15us) stays saturated.
Deferred QKV prep is emitted in 256-column quarters (~0.85 us of PE work)
so a single pop never overruns the per-window PE budget by more than the
one window of elasticity the 2-deep S psum ring provides.

The PE clock gate (HAM) only counts real matmuls as activity, so warmup is
~60 junk 128x128 matmuls on the identity (issued while the first DMA chunks
stream in), and 40 more junk matmuls after the last window keep the
clock warm through the final normalize -> projection -> DMA drain.

QCH=512 keeps every psum user at 1-2 banks: 4 banks S double-buffer +
2 banks PV accumulators + 2 banks for filler work (projection blocks and
the deferred QKV pieces), so fillers never contend with the S ring.
"""

import numpy as np
import ml_dtypes

B, N, D, H, DP = 2, 2048, 1024, 16, 64
SCALE = float(DP) ** 0.5
NCORES = 8
HC = H // NCORES            # heads per core = 2
E = HC * DP                 # per-core head-dim total = 128
QCH = 512                   # q columns handled per attention chunk
NQ = N // QCH               # 4
KB = N // 128               # 16 k blocks
DC = D // 128               # 8 contraction chunks for the qkv projection
NW = B * NQ * KB            # 128 attention windows

BF16 = ml_dtypes.bfloat16

_CACHE = {}


def _build_bass(with_bias=False):
    import concourse.bass as bass
    import concourse.mybir as mybir
    import concourse.tile as tile
    from concourse import bacc
    from concourse.masks import make_identity

    MM_DT = mybir.dt.bfloat16    # matmul input dtype
    P_DT = mybir.dt.bfloat16     # exp(S^T) storage dtype
    F32 = mybir.dt.float32

    # nonzero b_qkv is handled by an extra contraction chunk whose x^T rows
    # are [ones, 0...] and whose weight rows carry the bias (bias as matmul)
    DCX = DC + (1 if with_bias else 0)
    VAW = 130  # VA free width: 2 heads x [ones | V(64)]
    RING = 4   # P^T ring depth (PV runs at lag 2)
    nc = bacc.Bacc(None, target_bir_lowering=False)
    # x^T quarter-major: [B, quarter, partition, dc*512] (8KB/partition/DMA)
    xt = nc.dram_tensor("xt", [B, NQ, 128, DCX * 512], MM_DT, kind="ExternalInput")[:]
    # W_qkv partition-major: [partition, eb*dc*128] -> ONE 6KB-run transfer
    wsel = nc.dram_tensor("wsel", [128, 3 * DCX * 128], MM_DT, kind="ExternalInput")[:]
    wout = nc.dram_tensor("wout", [E, D], MM_DT, kind="ExternalInput")[:]
    # bf16 partials halve the output DMA; the host sums in fp32
    y = nc.dram_tensor("y", [B, N, D], MM_DT, kind="ExternalOutput")[:]

    with tile.TileContext(nc) as tc:
        with (
            tc.tile_pool(name="consts", bufs=1) as consts,
            tc.tile_pool(name="xtp", bufs=2) as xtp,
            tc.tile_pool(name="ptp", bufs=4) as ptp,
            tc.tile_pool(name="qkvp", bufs=2) as qkvp,
            tc.tile_pool(name="vap", bufs=2) as vap,
            tc.tile_pool(name="otp", bufs=2) as otp,
            tc.tile_pool(name="evacp", bufs=2) as evacp,
            tc.tile_pool(name="normp", bufs=2) as normp,
            # 8 psum banks total: paired-S 2x[128,2,512]f32 (4) +
            # pv accumulators 2x[65,512] (2) + filler scratch 2x[128,512] (2)
            tc.tile_pool(name="ps_s", bufs=2, space="PSUM") as ps_s,
            tc.tile_pool(name="ps_g", bufs=2, space="PSUM") as ps_g,
            tc.tile_pool(name="ps_y", bufs=2, space="PSUM") as ps_y,
        ):
            # SBUF layouts mirror the dram layouts (x^T quarter-major, W
            # eb-major) so every transfer is contiguous on BOTH sides:
            # 8KB runs per partition hit ~340 GB/s vs ~105 GB/s for the
            # 1KB-descriptor-run strided version.
            XTs = []
            for b in range(B):
                XTs.append(xtp.tile([128, NQ, DCX, 512], MM_DT, tag="xt", name="xt"))
            WS = consts.tile([128, 3, DCX, E], MM_DT)
            WOUT = consts.tile([128, D], MM_DT)

            def xt_dma(eng, b2, q):
                # full quarters only: slicing the free axis would split the
                # 8KB-contiguous runs into sub-1KB descriptors and drop the
                # transfer from ~270 GB/s to ~100 GB/s
                eng.dma_start(
                    out=XTs[b2][:, q],
                    in_=xt[b2, q].rearrange("p (dc n) -> p dc n", dc=DCX),
                )

            # Three independent DMA paths exist (sync HWDGE ring, scalar
            # HWDGE ring, gpsimd SWDGE); transfers FIFO-serialize per ring,
            # so the first S window's inputs are spread across all three:
            # scalar carries x quarter 0 + W_k, sync W_q, gpsimd W_v. The
            # WARM exp (which pulls in the ~1.3us exp table load) is
            # emitted mid-ring so the table is resident by S(0).
            # HBM bandwidth (~358 GB/s) is shared by all in-flight
            # transfers, so batch b1's 4MB rides BEHIND b0's quarters in
            # the same FIFO rings — b0 (all the first windows need) gets
            # the full bandwidth and lands ~13us instead of ~25us.
            WARM = consts.tile([1, 1], F32)
            nc.vector.memset(WARM, 0.0)
            wsr = wsel.rearrange("p (e dc f) -> p e dc f", e=3, dc=DCX)
            nc.sync.dma_start(out=WS[:, 0:2], in_=wsr[:, 0:2])   # W_q + W_k
            nc.sync.dma_start(out=WS[:, 2:3], in_=wsr[:, 2:3])   # W_v (late ok)
            xt_dma(nc.scalar, 0, 0)  # x b0 quarter 0
            nc.scalar.activation(
                out=WARM, in_=WARM, func=mybir.ActivationFunctionType.Exp
            )
            xt_dma(nc.scalar, 0, 1)
            xt_dma(nc.scalar, 0, 2)
            xt_dma(nc.scalar, 0, 3)
            nc.sync.dma_start(out=WOUT, in_=wout)
            # batch b1's 4MB is deliberately NOT issued here: it would
            # steal HBM bandwidth from b0's quarters (the whole startup
            # critical path). It is issued on gpsimd after the first
            # normalize (~28us), still ~50us before its first reader.

            IDENT = consts.tile([128, 128], MM_DT)
            make_identity(nc, IDENT)
            # HAM warmup: the PE clock gate only counts real matmuls (not
            # transposes) as activity. ~80 junk matmuls on the identity
            # bridge the DMA wait so the real prep matmuls run at 2.4 GHz.
            WARMPS = ps_g.tile([128, 128], F32, tag="g", name="warm_ps")
            for _ in range(60):
                nc.tensor.matmul(WARMPS, lhsT=IDENT, rhs=IDENT, start=True, stop=True)

            QKVTs, VAs = [], []
            fillers = []  # deferred projection sub-blocks (no deadline)
            for b in range(B):
                QKVTs.append(
                    [
                        qkvp.tile([128, N], MM_DT, tag=f"qkv{eb}", name=f"qkv{eb}")
                        for eb in range(3)
                    ]
                )
                # V chunks with a LEADING ones column: [1 | V_h0(64) | 1 | V_h1]
                VA = vap.tile([128, KB, VAW], MM_DT, tag="va", name="va")
                nc.gpsimd.memset(VA[:, :, 0:1], 1.0)
                nc.gpsimd.memset(VA[:, :, VAW // 2 : VAW // 2 + 1], 1.0)
                VAs.append(VA)

            def emit_qkv_quarter(b2, eb, qq, pool, split=False):
                # quarter of a projection block: one 256-col chunk of QKV^T.
                # split=True emits only the first half of the contraction
                # (4 matmuls ~ 0.43us, inside one window's PE slack) and
                # returns a continuation for the rest; the caller must run
                # it before any other allocation from the same psum ring.
                tag = "g" if pool is ps_g else "q"
                ps = pool.tile([128, 256], F32, tag=tag, name="ps_qkv")
                c0 = qq * 256
                qtr, o = qq // 2, (qq % 2) * 256

                def run(dcs, fin):
                    for dc in dcs:
                        nc.tensor.matmul(
                            ps,
                            lhsT=WS[:, eb, dc, :],
                            rhs=XTs[b2][:, qtr, dc, o : o + 256],
                            start=(dc == 0),
                            stop=(dc == DCX - 1),
                        )
                    if fin:
                        nc.vector.tensor_copy(
                            out=QKVTs[b2][eb][:, c0 : c0 + 256], in_=ps
                        )

                if split:
                    run(range(4), False)
                    return lambda: run(range(4, DCX), True)
                run(range(DCX), True)
                return None

            def emit_vtrans(b2, kc, pool):
                tag = "g" if pool is ps_g else "q"
                pst = pool.tile([128, 128], MM_DT, tag=tag, name="ps_vt")
                VT2 = QKVTs[b2][2]
                VA2 = VAs[b2]
                nc.tensor.transpose(
                    pst, VT2[:, kc * 128 : (kc + 1) * 128], IDENT
                )
                nc.vector.tensor_copy(out=VA2[:, kc, 1 : 1 + DP], in_=pst[:, 0:DP])
                nc.vector.tensor_copy(
                    out=VA2[:, kc, VAW // 2 + 1 : VAW // 2 + 1 + DP],
                    in_=pst[:, DP : 2 * DP],
                )

            # ---- deferred-prep schedule. Window index W counts kc windows
            # globally (16 per qh, 64 per batch). Each prep item carries the
            # last window index at which it may be emitted (one before its
            # first reader); pops happen after S(W)/PV(W-2), and the reader
            # S(W+1) is emitted after the pops of window W, so a deadline of
            # W is safe for readers in window W+1.
            def deadlines(base, items):
                return [(base + dl, it) for dl, it in items]

            # deadlines: S(0) needs only Q quarters 0-1 and K quarter 0, so
            # exactly those go negative (popped in the serial pre-loop);
            # everything else is spread 1-2 items per window just ahead of
            # its first reader: K qq read by S(2qq), V qq by vtrans(2qq)
            # (dl 2qq), vtrans kc by PV(kc) in window kc+2 (dl kc+1).
            prep = []
            for b2 in range(B):
                base = 64 * b2
                items = []
                for qq in range(2 * NQ):
                    qdl = -2 + (qq % 2) if qq < 2 else 16 * (qq // 2) - 6 + (qq % 2)
                    items.append(
                        (qdl,
                         lambda split=False, b2=b2, qq=qq:
                         emit_qkv_quarter(b2, 0, qq, ps_y, split))
                    )
                    items.append(
                        (-2 if qq == 0 else 2 * qq - 1,
                         lambda split=False, b2=b2, qq=qq:
                         emit_qkv_quarter(b2, 1, qq, ps_y, split))
                    )
                    items.append(
                        (2 * qq,
                         lambda split=False, b2=b2, qq=qq:
                         emit_qkv_quarter(b2, 2, qq, ps_y, split))
                    )
                for kc in range(KB):
                    items.append(
                        (kc + 1,
                         lambda split=False, b2=b2, kc=kc:
                         emit_vtrans(b2, kc, ps_y))
                    )
                prep.extend(deadlines(base, items))
            prep.sort(key=lambda it: it[0])

            # b=0 items that must precede S(0) run serially now
            while prep and prep[0][0] < 0:
                _, it = prep.pop(0)
                it()

            def emit_proj_block(spec, pool=None, use_scalar=False):
                b2, OT2, nb = spec
                pool = ps_y if pool is None else pool
                tag = "g" if pool is ps_g else ("s" if pool is ps_s else "q")
                ysb = evacp.tile([128, D], MM_DT, tag="y", name="ysb", bufs=4)
                for dc2 in range(D // 512):
                    py = pool.tile([128, 512], F32, tag=tag, name="py")
                    nc.tensor.matmul(
                        py,
                        lhsT=OT2[:, nb * 128 : (nb + 1) * 128],
                        rhs=WOUT[:, dc2 * 512 : (dc2 + 1) * 512],
                        start=True,
                        stop=True,
                    )
                    if use_scalar and dc2 == 1:
                        # drain phase only: ScalarE is done with exps, so
                        # splitting the evacuation halves the serial DVE
                        # cast chain on the critical tail
                        nc.scalar.activation(
                            out=ysb[:, dc2 * 512 : (dc2 + 1) * 512],
                            in_=py,
                            func=mybir.ActivationFunctionType.Copy,
                        )
                    else:
                        nc.vector.tensor_copy(
                            out=ysb[:, dc2 * 512 : (dc2 + 1) * 512], in_=py
                        )
                nc.sync.dma_start(
                    out=y[b2, nb * 128 : (nb + 1) * 128, :], in_=ysb
                )

            # ---- phase 2: attention, one flat stream of NW windows.
            OTs = [otp.tile([128, N], MM_DT, tag="ot", name="ot") for b in range(B)]
            PTmap = {}
            pvmap = {}

            def pv_mms(W2):
                b, qh, kc = W2 // 64, (W2 // 16) % 4, W2 % 16
                if kc == 0:
                    # allocate at first write, strictly after ALL of the
                    # previous qh's accesses to the evicted ring slots have
                    # been emitted (PV tail + normalize reads) — allocating
                    # any earlier races the ring reuse
                    pvmap[(b, qh)] = [
                        ps_g.tile([DP + 1, 512], F32, tag="g", name=f"pv{h}")
                        for h in range(HC)
                    ]
                pvs = pvmap[(b, qh)]
                PT = PTmap[(b, qh)]
                VA = VAs[b]
                for h in range(HC):
                    nc.tensor.matmul(
                        pvs[h],
                        lhsT=VA[
                            :, kc,
                            h * (VAW // 2) : h * (VAW // 2) + DP + 1,
                        ],
                        rhs=PT[:, kc % RING, h, :],
                        start=(kc == 0),
                        stop=(kc == KB - 1),
                    )

            def emit_norm(b, qh, use_scalar=False):
                # normalize: denominator row is psum partition 0 (leading
                # ones column). reciprocal straight off the psum denominator
                # row FIRST (the gpsimd broadcast then runs concurrently
                # with the pv evacuation copy), then broadcast, multiply
                # (rows 0..64 for base-partition alignment; row 0 unused),
                # DMA-shift into O^T.
                pvs = pvmap.pop((b, qh))
                OT = OTs[b]
                for h in range(HC):
                    pv = pvs[h]
                    rt = normp.tile([1, QCH], F32, tag="rt", name="rt")
                    nc.vector.reciprocal_approx_fast(out=rt, in_=pv[0:1, :])
                    ocp = normp.tile([DP + 1, QCH], F32, tag="ocp", name="ocp")
                    nc.vector.tensor_copy(out=ocp, in_=pv)
                    bc = normp.tile([DP + 1, QCH], F32, tag="bc", name="bc")
                    nc.gpsimd.partition_broadcast(bc, rt)
                    ots = normp.tile([DP + 1, QCH], MM_DT, tag="ots", name="ots")
                    nc.vector.tensor_mul(out=ots, in0=ocp, in1=bc)
                    nc.sync.dma_start(
                        out=OT[h * DP : (h + 1) * DP, qh * QCH : (qh + 1) * QCH],
                        in_=ots[1 : DP + 1, :],
                    )
                # queue this qh's projection blocks as fillers
                for nb in range(qh * QCH // 128, (qh + 1) * QCH // 128):
                    fillers.append(
                        (lambda pool=None, use_scalar=False, s=(b, OT, nb):
                         emit_proj_block(s, pool, use_scalar))
                    )
                if (b, qh) == (0, 0):
                    # b0 is resident; start streaming b1 now
                    for q2 in range(NQ):
                        xt_dma(nc.gpsimd, 1, q2)

            for W in range(NW + 2):
                b, qh, kc = W // 64, (W // 16) % 4, W % 16
                if W < NW:
                    if kc == 0:
                        PTmap[(b, qh)] = ptp.tile(
                            [128, RING, HC, 512], P_DT, tag="pt", name="pt"
                        )
                    # S FIRST: exp(W) is gated only on exp(W-1) and S(W),
                    # so nothing else may sit between them on the PE
                    PT = PTmap[(b, qh)]
                    QT, KT = QKVTs[b][0], QKVTs[b][1]
                    ps2 = ps_s.tile([128, HC, 512], F32, tag="s", name="s2")
                    q0 = qh * QCH
                    for h in range(HC):
                        nc.tensor.matmul(
                            ps2[:, h, :],
                            lhsT=KT[
                                h * DP : (h + 1) * DP,
                                kc * 128 : (kc + 1) * 128,
                            ],
                            rhs=QT[h * DP : (h + 1) * DP, q0 : q0 + 512],
                            start=True,
                            stop=True,
                        )
                    nc.scalar.activation(
                        out=PT[:, kc % RING, :, :],
                        in_=ps2,
                        func=mybir.ActivationFunctionType.Exp,
                        scale=1.0 / SCALE,
                    )
                if W >= 2:
                    pv_mms(W - 2)
                    if (W - 2) % 16 == 15:
                        emit_norm((W - 2) // 64, ((W - 2) // 16) % 4,
                                  use_scalar=(W - 2 == NW - 1))
                # mandatory deadline pops, then ONE opportunistic pop.
                popped = False
                while prep and prep[0][0] <= W:
                    prep.pop(0)[1]()
                    popped = True
                if not popped and W >= 1:
                    # prep first: its deadlines cluster in each batch's
                    # first qh, so draining it early smooths those windows;
                    # projection fillers have no deadline and soak up the
                    # idle back-half windows instead.
                    if prep and len(fillers) <= 12:
                        prep.pop(0)[1]()
                    elif fillers:
                        fillers.pop(0)(use_scalar=(W >= NW))
                        if len(fillers) > 10:
                            fillers.pop(0)(use_scalar=(W >= NW))

            # keep the PE clock warm through the final normalize chain so
            # the last projection blocks run at full clock
            WARMPS2 = ps_y.tile([128, 128], F32, tag="q", name="warm2")
            for _ in range(40):
                nc.tensor.matmul(WARMPS2, lhsT=IDENT, rhs=IDENT, start=True, stop=True)

            # drain remaining fillers, rotating through all three psum
            # pools (the S ring is free once the exps are done) to keep
            # 6 blocks in flight
            di = 0
            drain_pools = [ps_y, ps_g]
            while fillers:
                fillers.pop(0)(drain_pools[di % 2], use_scalar=True)
                di += 1
    nc.finalize()
    return nc


def _get_bass(with_bias=False):
    key = f"nc{int(with_bias)}"
    if key not in _CACHE:
        _CACHE[key] = _build_bass(with_bias)
    return _CACHE[key]


def _make_in_maps(x, W_qkv, b_qkv, W_out):
    """Shard the full inputs into the 8 per-core input dicts."""
    x = np.asarray(x, dtype=np.float32)
    W_qkv = np.asarray(W_qkv, dtype=np.float32)
    b_qkv = np.asarray(b_qkv, dtype=np.float32)
    W_out = np.asarray(W_out, dtype=np.float32)

    with_bias = bool(np.any(b_qkv))
    DCX = DC + (1 if with_bias else 0)
    # x^T per batch, shared by all cores (+ optional bias chunk rows)
    xtt = x.transpose(0, 2, 1)
    if with_bias:
        aug = np.zeros((B, 128, N), dtype=np.float32)
        aug[:, 0, :] = 1.0
        xtt = np.concatenate([xtt, aug], axis=1)
    # quarter-major: [B, 4, 128, DCX*512] so each quarter is one DMA with
    # 8KB contiguous per partition
    xq = (
        xtt.reshape(B, DCX, 128, NQ, 512)
        .transpose(0, 3, 2, 1, 4)
        .reshape(B, NQ, 128, DCX * 512)
    )
    xt = np.ascontiguousarray(xq).astype(BF16)

    in_maps = []
    for c in range(NCORES):
        heads = [HC * c + i for i in range(HC)]
        # W_qkv columns: head h occupies cols [h*3*DP, (h+1)*3*DP) as [q|k|v]
        qcols = [W_qkv[:, h * 3 * DP : h * 3 * DP + DP] for h in heads]
        kcols = [W_qkv[:, h * 3 * DP + DP : h * 3 * DP + 2 * DP] for h in heads]
        vcols = [W_qkv[:, h * 3 * DP + 2 * DP : h * 3 * DP + 3 * DP] for h in heads]
        wsel = np.concatenate(qcols + kcols + vcols, axis=1)  # [D, 3*E]
        if with_bias:
            bq = [b_qkv[h * 3 * DP : h * 3 * DP + DP] for h in heads]
            bk = [b_qkv[h * 3 * DP + DP : h * 3 * DP + 2 * DP] for h in heads]
            bv = [b_qkv[h * 3 * DP + 2 * DP : h * 3 * DP + 3 * DP] for h in heads]
            brow = np.concatenate(bq + bk + bv)  # [3*E]
            baug = np.zeros((128, 3 * E), dtype=np.float32)
            baug[0, :] = brow
            wsel = np.concatenate([wsel, baug], axis=0)
        # partition-major [128, 3*DCX*128]: one 6KB-contiguous-run transfer
        wq = (
            wsel.reshape(DCX, 128, 3, E)
            .transpose(1, 2, 0, 3)
            .reshape(128, 3 * DCX * E)
        )
        woutc = np.concatenate(
            [W_out[h * DP : (h + 1) * DP, :] for h in heads], axis=0
        )  # [E, D]
        in_maps.append(
            {
                "xt": xt,
                "wsel": np.ascontiguousarray(wq).astype(BF16),
                "wout": np.ascontiguousarray(woutc).astype(BF16),
            }
        )
    return in_maps, with_bias


def _run(in_maps, with_bias=False, trace=False):
    from concourse import bass_utils

    nc = _get_bass(with_bias)
    return bass_utils.run_bass_kernel_spmd(
        nc, in_maps, core_ids=list(range(NCORES)), trace=trace
    )


def kernel(x, W_qkv, b_qkv, W_out, b_out, _trace=False):
    in_maps, with_bias = _make_in_maps(x, W_qkv, b_qkv, W_out)
    res = _run(in_maps, with_bias=with_bias, trace=_trace)
    y = np.zeros((B, N, D), dtype=np.float32)
    for r in res.results:
        y += np.asarray(r["y"], dtype=np.float32)
    y += np.asarray(b_out, dtype=np.float32)
    _CACHE["last_result"] = res
    return y
